# revision 3
# baseline (speedup 1.0000x reference)
"""Trainium2 Bass kernel for nn_Block_34067680592489.

Computes, for B=32768 independent signals x[b] (length 256):
  mu,reg = small-CNN(x[b])      (conv5+avgpool4+softplus twice, linear, softplus)
  grad   = TtT x - x_b + reg * DtD x
  x_t    = x - gamma * grad,  gamma = softplus(gamma_p)
  out    = middle root of z^3 -(m+x_t) z^2 + (m x_t - 2 gm) z + gm m,  gm = gamma*mu

Device algorithm (per element, normalized to mass=1, s = (1+xt)/3):
  sqe = (s - 1/2)^2                          (ACT Square, table-free)
  c13 = 2/3*gph + 1/4,  gph = gamma*mu - 1/4 (per-row params from the CNN)
  hm  = sqe + c13   (= -p/3 > 1/12 always; never materialized)
  D'  = hm^3 - sqe*(sqe+gph)^2               (fused DVE op; = dm4/4 > 0)
  r   = 2*sqrt(hm) = Sqrt(4*sqe + 4*c13)     (ACT Sqrt, per-partition bias)
  irs = Rsqrt(D')                            (ACT)
  w   = (s-1/2)*(sqe+gph)*irs  (= -u)        (fused DVE op)
  at  = Arctan(w)                            (ACT)
  root= s - r*sin(at/3)                      (DVE odd-poly * r;  Pool adds s)

Sharding: pure data parallel over batch, 8 cores x 4096 rows.  x arrives
pre-transposed bf16 so the PE contraction dim is on partitions; x_b arrives
pre-scaled bf16 batch-major and enters PSUM via a single eye-stationary
identity matmul per tile (start=True resets the bank); the W_A|W_B matmuls
then accumulate on top as two 512-wide matmuls per tile.

Phase order: full CNN (exp/ln table) first, then main matmuls with per-chunk
Square/sqrt (sqrt table) + DMS, then an rsqrt block (abs_rsqrt table), then
arctan block (trig table), RGSIN + Pool add + DMA out.  Table-block
boundaries carry sync=True deps so the ACT engine never thrashes tables.
"""

import numpy as np

B_TOTAL = 32768
N = 256
N_CORES = 8
BC = B_TOTAL // N_CORES      # rows per core
TILES = BC // 128            # 32 batch tiles of 128
CT = 4                       # tiles per elementwise chunk
CHUNKS = TILES // CT         # 8
CF = CT * N                  # chunk free size (1024)

_PROG = {}


def _np_f32(a):
    return np.ascontiguousarray(np.asarray(a, dtype=np.float32))


def _conv_pool_mat(w, L):
    """(L/4, L) matrix implementing conv1d(k=5,pad=2) then avgpool4."""
    taps = np.asarray(w, np.float32).reshape(5)
    C = np.zeros((L, L), np.float32)
    for n in range(L):
        for k in range(5):
            m = n + k - 2
            if 0 <= m < L:
                C[n, m] = taps[k]
    P = np.zeros((L // 4, L), np.float32)
    for i in range(L // 4):
        P[i, 4 * i:4 * i + 4] = 0.25
    return (P @ C).astype(np.float32)


_CUSTOM_OPS = {}


def _get_custom_ops():
    """Register this kernel's fused custom-DVE ops (idempotent).

    DMS: D' = (sqe+c13)^3 - sqe*(sqe+gph)^2       (C0=c13, C1=gph)
    WU:  w  = ((s-1/2)*((s-1/2)^2+gph))*irs       (C0=gph, imm2=1/2)
    RGSIN: rg = (at*(s0+at^2*(s1+at^2*imm2)))*r   (odd sin(x/3) poly)
    """
    if _CUSTOM_OPS:
        return _CUSTOM_OPS
    import concourse.dve_ops as dops
    from concourse.dve_spec import (Spec, Src0, Src1, C0, C1, C2, sq,
                                    lower, _has_src1)
    from concourse.dve_uop import DveOpSpec

    def reg(name, spec):
        if name in dops._SUB_OPCODE_FOR_NAME:
            return next(o for o in dops.OPS if o.name == name)
        row = dops._CUSTOM_DVE_ROW_BASE + len(dops.OPS)
        assert row < 0x20
        dops._SUB_OPCODE_FOR_NAME[name] = row
        shas = {}
        for ver in ("v3", "v4"):
            u = lower(spec, ver=ver)
            shas[ver] = DveOpSpec(name=name, opcode=row, uops=u,
                                  rd1_en=_has_src1(spec)).sha(ver)
        op = dops.DveOp(name, spec, subdim=False, uops_sha=shas)
        dops.OPS.append(op)
        dops.CUSTOM_DVE_SPECS[name] = spec
        return op

    import numpy as np_

    _h = Src0 + C0
    _CUSTOM_OPS['DMS'] = reg('ANT_K_DMS', Spec(
        body=(sq(_h) * _h) - Src0 * sq(Src0 + C1),
        reference=lambda in0, in1, s0, s1, imm2:
            (((in0 + s0) ** 2 * (in0 + s0))
             - in0 * (in0 + s1) ** 2).astype(np_.float32),
    ))
    _em = Src0 - C2
    _CUSTOM_OPS['WU'] = reg('ANT_K_WU', Spec(
        body=(_em * (sq(_em) + C0)) * Src1,
        reference=lambda in0, in1, s0, s1, imm2:
            (((in0 - imm2) * ((in0 - imm2) ** 2 + s0)) * in1
             ).astype(np_.float32),
    ))
    _a2 = sq(Src0)
    _CUSTOM_OPS['RGSIN'] = reg('ANT_K_RGSIN', Spec(
        body=(Src0 * (C0 + _a2 * (C1 + _a2 * C2))) * Src1,
        reference=lambda in0, in1, s0, s1, imm2:
            ((in0 * (s0 + in0 * in0 * (s1 + in0 * in0 * imm2))) * in1
             ).astype(np_.float32),
    ))
    return _CUSTOM_OPS


_TABLES_PATCHED = False


def _patch_act_tables():
    """Restrict ACT table-set choice to the sets this kernel uses so the
    chooser binds Exp/Ln -> natural_log_exp_and_others, Sqrt ->
    sqrt_and_others, Rsqrt -> reciprocal_sqrt_and_small, Arctan ->
    trig_and_small (Square is in every set and never forces a load)."""
    global _TABLES_PATCHED
    if _TABLES_PATCHED:
        return
    import concourse.bacc as bacc
    keep = {'natural_log_exp_and_others', 'sqrt_and_others',
            'abs_reciprocal_sqrt_and_small', 'trig_and_small'}
    orig = bacc.get_activation_tables

    def patched(arch):
        t = orig(arch)
        return {k: (v if k in keep else set()) for k, v in t.items()}

    bacc.get_activation_tables = patched
    _TABLES_PATCHED = True


def _build_program():
    import concourse.bacc as bacc
    import concourse.tile as tile
    import concourse.mybir as mybir
    from concourse.tile import add_dep_helper
    _patch_act_tables()

    dt = mybir.dt
    f32 = dt.float32
    bf16 = dt.bfloat16
    Alu = mybir.AluOpType
    AF = mybir.ActivationFunctionType
    odt = bf16

    COPS = _get_custom_ops()
    nc = bacc.Bacc("TRN2", target_bir_lowering=False, debug=False,
                   num_devices=N_CORES)

    XT = nc.dram_tensor("xt", (256, BC), bf16, kind="ExternalInput")
    XB = nc.dram_tensor("xb", (BC, 256), bf16, kind="ExternalInput")
    EYE = nc.dram_tensor("eye", (128, 128), bf16, kind="ExternalInput")
    WM = nc.dram_tensor("wm", (256, 512), bf16, kind="ExternalInput")
    M1T = nc.dram_tensor("m1t", (256, 128), bf16, kind="ExternalInput")
    M2BD = nc.dram_tensor("m2bd", (128, 32), bf16, kind="ExternalInput")
    LWBD = nc.dram_tensor("lwbd", (128, 2), bf16, kind="ExternalInput")
    B2V = nc.dram_tensor("b2v", (128, 1), f32, kind="ExternalInput")
    B3V = nc.dram_tensor("b3v", (128, 1), f32, kind="ExternalInput")
    LBM = nc.dram_tensor("lbm", (128, 1), f32, kind="ExternalInput")
    LBR = nc.dram_tensor("lbr", (128, 1), f32, kind="ExternalInput")
    GSC = nc.dram_tensor("gsc", (128, 1), f32, kind="ExternalInput")
    OUT = nc.dram_tensor("out", (BC, 256), odt, kind="ExternalOutput")

    NSG = 2                          # supergroups
    GPS = CHUNKS // NSG              # groups per supergroup

    with tile.TileContext(nc) as tc:
        with (
            tc.tile_pool(name="const", bufs=1) as cpool,
            tc.tile_pool(name="so", bufs=CHUNKS) as sopool,
            tc.tile_pool(name="sq", bufs=4) as sqpool,
            tc.tile_pool(name="dp", bufs=CHUNKS) as dppool,
            tc.tile_pool(name="wv", bufs=3) as wvpool,
            tc.tile_pool(name="rr", bufs=CHUNKS) as rrpool,
            tc.tile_pool(name="oo", bufs=3) as oopool,
            tc.tile_pool(name="pm", bufs=4, space="PSUM") as pmpool,
            tc.tile_pool(name="pc1", bufs=2, space="PSUM") as pc1pool,
            tc.tile_pool(name="pc2", bufs=1, space="PSUM") as pc2pool,
            tc.tile_pool(name="pc3", bufs=1, space="PSUM") as pc3pool,
        ):
            # ---- constants into SBUF ----
            wm = cpool.tile([128, 2, 512], bf16)
            m1t = cpool.tile([128, 2, 128], bf16)
            m2bd = cpool.tile([128, 32], bf16)
            lwbd = cpool.tile([128, 2], bf16)
            b2v = cpool.tile([128, 1], f32)
            b3v = cpool.tile([128, 1], f32)
            lbm = cpool.tile([128, 1], f32)
            lbr = cpool.tile([128, 1], f32)
            gsc = cpool.tile([128, 1], f32)
            spE = cpool.tile([128, 2 * TILES], f32)
            sp = cpool.tile([128, 2 * TILES], f32)
            gph = cpool.tile([128, TILES], f32)
            c13p = cpool.tile([128, TILES], f32)
            c13x4 = cpool.tile([128, TILES], f32)
            eye = cpool.tile([128, 128], bf16)
            nc.sync.dma_start(eye[:], EYE[:])
            cm16 = cpool.tile([128, 1], f32)
            nc.vector.memset(cm16[:], -0.5)
            for k in range(2):
                nc.sync.dma_start(m1t[:, k, :], M1T[128 * k:128 * (k + 1), :])
            nc.sync.dma_start(m2bd[:], M2BD[:])
            nc.sync.dma_start(lwbd[:], LWBD[:])
            nc.sync.dma_start(b2v[:], B2V[:])
            nc.sync.dma_start(b3v[:], B3V[:])
            nc.sync.dma_start(lbm[:], LBM[:])
            nc.sync.dma_start(lbr[:], LBR[:])
            nc.sync.dma_start(gsc[:], GSC[:])

            s_chunks = [sopool.tile([128, CF], f32, tag="so", name=f"s{c}")
                        for c in range(CHUNKS)]
            sq_chunks = [None] * CHUNKS
            dp_chunks = [None] * CHUNKS
            w_chunks = [None] * CHUNKS
            r_chunks = [None] * CHUNKS

            with (
                tc.tile_pool(name="xt", bufs=1) as xtpool,
                tc.tile_pool(name="cnn", bufs=2) as cnnpool,
            ):
                # ---- inputs ----
                xt_sb = xtpool.tile([128, 2, BC], bf16)
                xb_sb = xtpool.tile([128, TILES, 256], bf16)
                XBv = XB[:].rearrange("(t p) n -> p t n", p=128)
                for qq in range(4):
                    qsl = slice(BC // 4 * qq, BC // 4 * (qq + 1))
                    tsl = slice(TILES // 4 * qq, TILES // 4 * (qq + 1))
                    for k in range(2):
                        nc.sync.dma_start(xt_sb[:, k, qsl],
                                          XT[128 * k:128 * (k + 1), qsl])
                    nc.gpsimd.dma_start(xb_sb[:, tsl, :], XBv[:, tsl, :])
                    if qq == 0:
                        for k in range(2):
                            nc.scalar.dma_start(wm[:, k, :],
                                                WM[128 * k:128 * (k + 1), :])

                sp_insts = []
                spEv = spE[:].rearrange("p (t c) -> p c t", c=2)
                spv = sp[:].rearrange("p (t c) -> p c t", c=2)
                p3 = pc3pool.tile([128, 2 * TILES], f32)

                # ======== CNN phase: both supergroups, exp/ln table ========
                for sg in range(NSG):
                    gs_range = range(GPS * sg, GPS * (sg + 1))
                    p2 = pc2pool.tile([128, 512], f32, tag="p2",
                                      name=f"p2sg{sg}")
                    for q, g in enumerate(gs_range):
                        sl = slice(512 * g, 512 * (g + 1))
                        p1 = pc1pool.tile([128, 512], f32, tag="p1",
                                          name=f"p1g{g}")
                        nc.tensor.matmul(p1[:], m1t[:, 0, :],
                                         xt_sb[:, 0, sl],
                                         start=True, stop=False)
                        nc.tensor.matmul(p1[:], m1t[:, 1, :],
                                         xt_sb[:, 1, sl],
                                         start=False, stop=True)
                        eh1 = cnnpool.tile([128, 512], f32, tag="eh1",
                                           name=f"eh1g{g}")
                        nc.scalar.activation(eh1[:], p1[:], AF.Exp,
                                             bias=b2v[:])
                        h1s = cnnpool.tile([128, 512], bf16, tag="h1s",
                                           name=f"h1sg{g}")
                        nc.scalar.activation(h1s[:], eh1[:], AF.Ln, bias=1.0)
                        nc.tensor.matmul(p2[32 * q:32 * (q + 1), :],
                                         m2bd[:], h1s[:],
                                         start=True, stop=True,
                                         tile_position=(0, 32 * q),
                                         skip_group_check=True)
                    eh2 = cnnpool.tile([128, 512], f32, tag="eh2",
                                       name=f"eh2sg{sg}")
                    nc.scalar.activation(eh2[:], p2[:], AF.Exp,
                                         bias=b3v[:])
                    h2s = cnnpool.tile([128, 512], bf16, tag="h2s",
                                       name=f"h2ssg{sg}")
                    nc.scalar.activation(h2s[:], eh2[:], AF.Ln, bias=1.0)
                    for q, g in enumerate(gs_range):
                        for i in range(4):
                            t = 4 * g + i
                            nc.tensor.matmul(
                                p3[:, 2 * t:2 * t + 2],
                                h2s[32 * q:32 * (q + 1),
                                    128 * i:128 * (i + 1)],
                                lwbd[32 * q:32 * (q + 1), :],
                                start=True, stop=True,
                                tile_position=(32 * q, 0),
                                skip_group_check=True)

                    sgt = slice(4 * GPS * sg, 4 * GPS * (sg + 1))
                    sgs = slice(8 * GPS * sg, 8 * GPS * (sg + 1))
                    nc.scalar.activation(spEv[:, 0, sgt],
                                         p3[:, sgs].rearrange(
                                             "p (t c) -> p c t", c=2)[:, 0, :],
                                         AF.Exp, bias=lbm[:])
                    nc.scalar.activation(spEv[:, 1, sgt],
                                         p3[:, sgs].rearrange(
                                             "p (t c) -> p c t", c=2)[:, 1, :],
                                         AF.Exp, bias=lbr[:])
                    sp_i = nc.scalar.activation(sp[:, sgs], spE[:, sgs],
                                                AF.Ln, bias=1.0)
                    sp_insts.append(sp_i)
                    nc.vector.tensor_scalar(gph[:, sgt], spv[:, 0, sgt],
                                            gsc[:], -0.25,
                                            Alu.mult, Alu.add)
                    nc.vector.tensor_scalar(c13p[:, sgt], gph[:, sgt],
                                            2.0 / 3.0, 0.25,
                                            Alu.mult, Alu.add)
                    nc.vector.tensor_scalar(c13x4[:, sgt], gph[:, sgt],
                                            8.0 / 3.0, 1.0,
                                            Alu.mult, Alu.add)

                # ======== main phase: matmuls + per-chunk s/sqe/DMS/sqrt ====
                first_sqrt = None
                last_sqrt_blk = None
                for c in range(CHUNKS):
                    s_c = s_chunks[c]
                    pms = []
                    # identity moves first: eye-stationary, one 256-wide
                    # matmul per tile; start=True resets the whole bank so
                    # the 512-wide W-matmuls below accumulate on a clean
                    # B half.
                    for i in range(CT):
                        t = CT * c + i
                        pm = pmpool.tile([128, 512], f32, tag="pm",
                                         name=f"pm{t}")
                        pms.append(pm)
                        nc.tensor.matmul(pm[:, 0:256], eye[:],
                                         xb_sb[:, t, :],
                                         start=True, stop=False,
                                         skip_group_check=True)
                    for i in range(CT):
                        t = CT * c + i
                        tsl = slice(128 * t, 128 * (t + 1))
                        pm = pms[i]
                        for k in range(2):
                            nc.tensor.matmul(
                                pm[:, 0:512],
                                xt_sb[:, k, tsl], wm[:, k, :],
                                start=False, stop=(k == 1),
                                skip_group_check=True)
                    # elementwise: td, s, per chunk Square, DMS, sqrt
                    for i in range(CT):
                        t = CT * c + i
                        pm = pms[i]
                        osl = slice(256 * i, 256 * (i + 1))
                        td = wvpool.tile([128, 256], f32, tag="td",
                                         name=f"td{t}")
                        nc.vector.tensor_scalar(
                            td[:], pm[:, 256:512],
                            spv[:, 1, t:t + 1], None, Alu.mult)
                        nc.vector.scalar_tensor_tensor(
                            s_c[:, osl], pm[:, 0:256], 1.0 / 3.0,
                            td[:], Alu.add, Alu.add)
                    sqe = sqpool.tile([128, CF], f32, tag="sq",
                                      name=f"sq{c}")
                    sq_chunks[c] = sqe
                    nc.scalar.activation(sqe[:], s_c[:], AF.Square,
                                         bias=cm16[:])
                    dp = dppool.tile([128, CF], f32, tag="dp",
                                     name=f"dp{c}")
                    dp_chunks[c] = dp
                    for i in range(CT):
                        t = CT * c + i
                        osl = slice(256 * i, 256 * (i + 1))
                        nc.vector._custom_dve(
                            COPS['DMS'], out=dp[:, osl],
                            in0=sqe[:, osl],
                            s0=c13p[:, t:t + 1], s1=gph[:, t:t + 1])
                    r = rrpool.tile([128, CF], f32, tag="rr", name=f"r{c}")
                    r_chunks[c] = r
                    for i in range(CT):
                        t = CT * c + i
                        osl = slice(256 * i, 256 * (i + 1))
                        sq_i = nc.scalar.activation(r[:, osl],
                                                    sqe[:, osl],
                                                    AF.Sqrt,
                                                    bias=c13x4[:, t:t + 1],
                                                    scale=4.0)
                        if first_sqrt is None:
                            first_sqrt = sq_i
                            for spi in sp_insts:
                                add_dep_helper(sq_i.ins, spi.ins, sync=True,
                                               reason="sqrt block after NLE")
                        else:
                            add_dep_helper(sq_i.ins, last_sqrt_blk.ins,
                                           sync=False,
                                           reason="chain sqrt block")
                        last_sqrt_blk = sq_i

            # ---- rsqrt block (abs_rsqrt table), WU on DVE ----
            last_rsq = None
            for c in range(CHUNKS):
                dp = dp_chunks[c]
                irs_i = nc.scalar.activation(dp[:], dp[:],
                                             AF.Abs_reciprocal_sqrt)
                if last_rsq is None:
                    add_dep_helper(irs_i.ins, last_sqrt_blk.ins, sync=True,
                                   reason="absrsqrt block after sqrt block")
                else:
                    add_dep_helper(irs_i.ins, last_rsq.ins, sync=False,
                                   reason="chain rsqrt block")
                last_rsq = irs_i
                w = wvpool.tile([128, CF], f32, tag="wv", name=f"w{c}")
                w_chunks[c] = w
                for i in range(CT):
                    t = CT * c + i
                    osl = slice(256 * i, 256 * (i + 1))
                    nc.vector._custom_dve(
                        COPS['WU'], out=w[:, osl], in0=s_chunks[c][:, osl],
                        in1=dp[:, osl], s0=gph[:, t:t + 1], imm2=0.5)

            # ---- trig block: arctan, RGSIN, add (Pool), DMA out ----
            last_at = None
            for c in range(CHUNKS):
                w = w_chunks[c]
                at_i = nc.scalar.activation(w[:], w[:], AF.Arctan)
                if last_at is None:
                    add_dep_helper(at_i.ins, last_rsq.ins, sync=True,
                                   reason="trig block after rsqrt block")
                else:
                    add_dep_helper(at_i.ins, last_at.ins, sync=False,
                                   reason="chain trig block")
                last_at = at_i
                rg = sqpool.tile([128, CF], f32, tag="sq", name=f"rg{c}")
                nc.vector._custom_dve(
                    COPS['RGSIN'], out=rg[:], in0=w[:],
                    in1=r_chunks[c][:],
                    s0=-1.0 / 3.0, s1=1.0 / 162.0, imm2=-1.0 / 29160.0)
                ot = oopool.tile([128, CF], odt, tag="oo", name=f"o{c}")
                nc.gpsimd.tensor_tensor(ot[:], rg[:], s_chunks[c][:],
                                        Alu.add)
                dview = OUT[512 * c:512 * (c + 1), :].rearrange(
                    "(tt p) n -> p tt n", p=128)
                nc.sync.dma_start(
                    dview, ot[:].rearrange("p (tt n) -> p tt n", n=256))

    nc.compile()
    return nc


def _get_program():
    key = (B_TOTAL, N, N_CORES)
    if key not in _PROG:
        _PROG[key] = _build_program()
    return _PROG[key]


def _host_prep(inputs):
    import ml_dtypes
    bf = ml_dtypes.bfloat16
    x = _np_f32(inputs['x']).reshape(B_TOTAL, N)
    x_b = _np_f32(inputs['x_b']).reshape(B_TOTAL, N)
    m = float(np.asarray(inputs['mass']).reshape(-1)[0])
    gp = float(np.asarray(inputs['gamma_p']).reshape(-1)[0])
    gamma = float(np.log1p(np.exp(gp))) if gp < 30 else gp
    TtT = _np_f32(inputs['TtT'])
    DtD = _np_f32(inputs['DtD'])

    W_A = ((np.eye(N, dtype=np.float32) - np.float32(gamma) * TtT.T)
           / np.float32(3.0 * m)).astype(np.float32)
    W_B = (-np.float32(gamma) * DtD.T / np.float32(3.0 * m)).astype(np.float32)
    WM = np.concatenate([W_A, W_B], axis=1).astype(bf)          # (256,512)

    M1s, M2s, lws = {}, {}, {}
    for tag in ('mu', 'reg'):
        M1s[tag] = _conv_pool_mat(inputs['w2_' + tag], 256)      # (64,256)
        M2s[tag] = _conv_pool_mat(inputs['w3_' + tag], 64)       # (16,64)
        lws[tag] = _np_f32(inputs['lw_' + tag]).reshape(16)
    M1cat = np.concatenate([M1s['mu'], M1s['reg']], axis=0)      # (128,256)
    M1T = np.ascontiguousarray(M1cat.T).astype(bf)               # (256,128)
    M2BD = np.zeros((128, 32), np.float32)
    M2BD[0:64, 0:16] = M2s['mu'].T
    M2BD[64:128, 16:32] = M2s['reg'].T
    M2BD = M2BD.astype(bf)
    LWBD1 = np.zeros((32, 2), np.float32)
    LWBD1[0:16, 0] = lws['mu']
    LWBD1[16:32, 1] = lws['reg']
    LWBD = np.tile(LWBD1, (4, 1)).astype(bf)                     # (128,2)

    def sc(name):
        return float(np.asarray(inputs[name]).reshape(-1)[0])

    B2V = np.full((128, 1), sc('b2_mu'), np.float32)
    B2V[64:] = sc('b2_reg')
    B3V1 = np.full((32, 1), sc('b3_mu'), np.float32)
    B3V1[16:] = sc('b3_reg')
    B3V = np.tile(B3V1, (4, 1))                                  # (128,1)
    LBM = np.full((128, 1), sc('lb_mu'), np.float32)
    LBR = np.full((128, 1), sc('lb_reg'), np.float32)
    GSC = np.full((128, 1), gamma / (m * m), np.float32)

    EYEM = np.eye(128, dtype=np.float32).astype(bf)
    consts = dict(wm=WM, m1t=M1T, m2bd=M2BD, lwbd=LWBD, eye=EYEM,
                  b2v=B2V, b3v=B3V, lbm=LBM, lbr=LBR, gsc=GSC)

    xb3 = (np.float32(gamma / (3.0 * m)) * x_b).astype(bf)
    xbf = x.astype(bf)
    in_maps = []
    for c in range(N_CORES):
        rows = slice(BC * c, BC * (c + 1))
        im = dict(consts)
        im['xt'] = np.ascontiguousarray(xbf[rows].T)
        im['xb'] = np.ascontiguousarray(xb3[rows])
        in_maps.append(im)
    return in_maps, m


def kernel(**inputs) -> np.ndarray:
    from concourse import bass_utils
    nc = _get_program()
    in_maps, m = _host_prep(inputs)
    res = bass_utils.run_bass_kernel_spmd(nc, in_maps,
                                          core_ids=list(range(N_CORES)))
    out = np.concatenate([res.results[c]['out'] for c in range(N_CORES)],
                         axis=0).astype(np.float32)
    if m != 1.0:
        out = (np.float32(m) * out).astype(np.float32)
    return np.ascontiguousarray(out.reshape(B_TOTAL, 1, N))


# revision 4
# speedup vs baseline: 1.0309x; 1.0309x over previous
"""Trainium2 Bass kernel for nn_Block_34067680592489.

Computes, for B=32768 independent signals x[b] (length 256):
  mu,reg = small-CNN(x[b])      (conv5+avgpool4+softplus twice, linear, softplus)
  grad   = TtT x - x_b + reg * DtD x
  x_t    = x - gamma * grad,  gamma = softplus(gamma_p)
  out    = middle root of z^3 -(m+x_t) z^2 + (m x_t - 2 gm) z + gm m,  gm = gamma*mu

Device algorithm (per element, normalized to mass=1, s = (1+xt)/3):
  sqe = (s - 1/2)^2                          (ACT Square, table-free)
  c13 = 2/3*gph + 1/4,  gph = gamma*mu - 1/4 (per-row params from the CNN)
  hm  = sqe + c13   (= -p/3 > 1/12 always; never materialized)
  D'  = hm^3 - sqe*(sqe+gph)^2               (fused DVE op; = dm4/4 > 0)
  r   = 2*sqrt(hm) = Sqrt(4*sqe + 4*c13)     (ACT Sqrt, per-partition bias)
  irs = Rsqrt(D')                            (ACT)
  w   = (s-1/2)*(sqe+gph)*irs  (= -u)        (fused DVE op)
  at  = Arctan(w)                            (ACT)
  root= s - r*sin(at/3)                      (DVE odd-poly * r;  Pool adds s)

Sharding: pure data parallel over batch, 8 cores x 4096 rows.  x arrives
pre-transposed bf16 so the PE contraction dim is on partitions; x_b arrives
pre-scaled bf16 batch-major and enters PSUM via a single eye-stationary
identity matmul per tile (start=True resets the bank); the W_A|W_B matmuls
then accumulate on top as two 512-wide matmuls per tile.

Phase order: full CNN (exp/ln table) first, then main matmuls with per-chunk
Square/sqrt (sqrt table) + DMS, then an rsqrt block (abs_rsqrt table), then
arctan block (trig table), RGSIN + Pool add + DMA out.  Table-block
boundaries carry sync=True deps so the ACT engine never thrashes tables.
"""

import numpy as np

B_TOTAL = 32768
N = 256
N_CORES = 8
BC = B_TOTAL // N_CORES      # rows per core
TILES = BC // 128            # 32 batch tiles of 128
CT = 4                       # tiles per elementwise chunk
CHUNKS = TILES // CT         # 8
CF = CT * N                  # chunk free size (1024)

_PROG = {}


def _np_f32(a):
    return np.ascontiguousarray(np.asarray(a, dtype=np.float32))


def _conv_pool_mat(w, L):
    """(L/4, L) matrix implementing conv1d(k=5,pad=2) then avgpool4."""
    taps = np.asarray(w, np.float32).reshape(5)
    C = np.zeros((L, L), np.float32)
    for n in range(L):
        for k in range(5):
            m = n + k - 2
            if 0 <= m < L:
                C[n, m] = taps[k]
    P = np.zeros((L // 4, L), np.float32)
    for i in range(L // 4):
        P[i, 4 * i:4 * i + 4] = 0.25
    return (P @ C).astype(np.float32)


_CUSTOM_OPS = {}


def _get_custom_ops():
    """Register this kernel's fused custom-DVE ops (idempotent).

    DMS: D' = (sqe+c13)^3 - sqe*(sqe+gph)^2       (C0=c13, C1=gph)
    WU:  w  = ((s-1/2)*((s-1/2)^2+gph))*irs       (C0=gph, imm2=1/2)
    RGSIN: rg = (at*(s0+at^2*(s1+at^2*imm2)))*r   (odd sin(x/3) poly)
    """
    if _CUSTOM_OPS:
        return _CUSTOM_OPS
    import concourse.dve_ops as dops
    from concourse.dve_spec import (Spec, Src0, Src1, C0, C1, C2, sq,
                                    lower, _has_src1)
    from concourse.dve_uop import DveOpSpec

    def reg(name, spec):
        if name in dops._SUB_OPCODE_FOR_NAME:
            return next(o for o in dops.OPS if o.name == name)
        row = dops._CUSTOM_DVE_ROW_BASE + len(dops.OPS)
        assert row < 0x20
        dops._SUB_OPCODE_FOR_NAME[name] = row
        shas = {}
        for ver in ("v3", "v4"):
            u = lower(spec, ver=ver)
            shas[ver] = DveOpSpec(name=name, opcode=row, uops=u,
                                  rd1_en=_has_src1(spec)).sha(ver)
        op = dops.DveOp(name, spec, subdim=False, uops_sha=shas)
        dops.OPS.append(op)
        dops.CUSTOM_DVE_SPECS[name] = spec
        return op

    import numpy as np_

    _h = Src0 + C0
    _CUSTOM_OPS['DMS'] = reg('ANT_K_DMS', Spec(
        body=(sq(_h) * _h) - Src0 * sq(Src0 + C1),
        reference=lambda in0, in1, s0, s1, imm2:
            (((in0 + s0) ** 2 * (in0 + s0))
             - in0 * (in0 + s1) ** 2).astype(np_.float32),
    ))
    _em = Src0 - C2
    _CUSTOM_OPS['WU'] = reg('ANT_K_WU', Spec(
        body=(_em * (sq(_em) + C0)) * Src1,
        reference=lambda in0, in1, s0, s1, imm2:
            (((in0 - imm2) * ((in0 - imm2) ** 2 + s0)) * in1
             ).astype(np_.float32),
    ))
    _a2 = sq(Src0)
    _CUSTOM_OPS['RGSIN'] = reg('ANT_K_RGSIN', Spec(
        body=(Src0 * (C0 + _a2 * (C1 + _a2 * C2))) * Src1,
        reference=lambda in0, in1, s0, s1, imm2:
            ((in0 * (s0 + in0 * in0 * (s1 + in0 * in0 * imm2))) * in1
             ).astype(np_.float32),
    ))
    return _CUSTOM_OPS


_TABLES_PATCHED = False


def _patch_act_tables():
    """Restrict ACT table-set choice to the sets this kernel uses so the
    chooser binds Exp/Ln -> natural_log_exp_and_others, Sqrt ->
    sqrt_and_others, Rsqrt -> reciprocal_sqrt_and_small, Arctan ->
    trig_and_small (Square is in every set and never forces a load)."""
    global _TABLES_PATCHED
    if _TABLES_PATCHED:
        return
    import concourse.bacc as bacc
    keep = {'natural_log_exp_and_others', 'sqrt_and_others',
            'abs_reciprocal_sqrt_and_small', 'trig_and_small'}
    orig = bacc.get_activation_tables

    def patched(arch):
        t = orig(arch)
        return {k: (v if k in keep else set()) for k, v in t.items()}

    bacc.get_activation_tables = patched
    _TABLES_PATCHED = True


def _build_program():
    import concourse.bacc as bacc
    import concourse.tile as tile
    import concourse.mybir as mybir
    from concourse.tile import add_dep_helper
    _patch_act_tables()

    dt = mybir.dt
    f32 = dt.float32
    bf16 = dt.bfloat16
    Alu = mybir.AluOpType
    AF = mybir.ActivationFunctionType
    odt = bf16

    COPS = _get_custom_ops()
    nc = bacc.Bacc("TRN2", target_bir_lowering=False, debug=False,
                   num_devices=N_CORES)

    XT = nc.dram_tensor("xt", (256, BC), bf16, kind="ExternalInput")
    XB = nc.dram_tensor("xb", (BC, 256), bf16, kind="ExternalInput")
    EYE = nc.dram_tensor("eye", (128, 128), bf16, kind="ExternalInput")
    WM = nc.dram_tensor("wm", (256, 512), bf16, kind="ExternalInput")
    M1T = nc.dram_tensor("m1t", (256, 128), bf16, kind="ExternalInput")
    M2BD = nc.dram_tensor("m2bd", (128, 32), bf16, kind="ExternalInput")
    LWBD = nc.dram_tensor("lwbd", (128, 2), bf16, kind="ExternalInput")
    B2V = nc.dram_tensor("b2v", (128, 1), f32, kind="ExternalInput")
    B3V = nc.dram_tensor("b3v", (128, 1), f32, kind="ExternalInput")
    LBM = nc.dram_tensor("lbm", (128, 1), f32, kind="ExternalInput")
    LBR = nc.dram_tensor("lbr", (128, 1), f32, kind="ExternalInput")
    GSC = nc.dram_tensor("gsc", (128, 1), f32, kind="ExternalInput")
    OUT = nc.dram_tensor("out", (BC, 256), odt, kind="ExternalOutput")

    NSG = 2                          # supergroups
    GPS = CHUNKS // NSG              # groups per supergroup

    with tile.TileContext(nc) as tc:
        with (
            tc.tile_pool(name="const", bufs=1) as cpool,
            tc.tile_pool(name="so", bufs=CHUNKS) as sopool,
            tc.tile_pool(name="sq", bufs=4) as sqpool,
            tc.tile_pool(name="dp", bufs=CHUNKS) as dppool,
            tc.tile_pool(name="wv", bufs=3) as wvpool,
            tc.tile_pool(name="rr", bufs=CHUNKS) as rrpool,
            tc.tile_pool(name="oo", bufs=3) as oopool,
            tc.tile_pool(name="pm", bufs=4, space="PSUM") as pmpool,
            tc.tile_pool(name="pc1", bufs=2, space="PSUM") as pc1pool,
            tc.tile_pool(name="pc2", bufs=1, space="PSUM") as pc2pool,
            tc.tile_pool(name="pc3", bufs=1, space="PSUM") as pc3pool,
        ):
            # ---- constants into SBUF ----
            wm = cpool.tile([128, 2, 512], bf16)
            m1t = cpool.tile([128, 2, 128], bf16)
            m2bd = cpool.tile([128, 32], bf16)
            lwbd = cpool.tile([128, 2], bf16)
            b2v = cpool.tile([128, 1], f32)
            b3v = cpool.tile([128, 1], f32)
            lbm = cpool.tile([128, 1], f32)
            lbr = cpool.tile([128, 1], f32)
            gsc = cpool.tile([128, 1], f32)
            spE = cpool.tile([128, 2 * TILES], f32)
            sp = cpool.tile([128, 2 * TILES], f32)
            gph = cpool.tile([128, TILES], f32)
            c13p = cpool.tile([128, TILES], f32)
            c13x4 = cpool.tile([128, TILES], f32)
            eye = cpool.tile([128, 128], bf16)
            nc.sync.dma_start(eye[:], EYE[:])
            cm16 = cpool.tile([128, 1], f32)
            nc.vector.memset(cm16[:], -0.5)
            for k in range(2):
                nc.sync.dma_start(m1t[:, k, :], M1T[128 * k:128 * (k + 1), :])
            nc.sync.dma_start(m2bd[:], M2BD[:])
            nc.sync.dma_start(lwbd[:], LWBD[:])
            nc.sync.dma_start(b2v[:], B2V[:])
            nc.sync.dma_start(b3v[:], B3V[:])
            nc.sync.dma_start(lbm[:], LBM[:])
            nc.sync.dma_start(lbr[:], LBR[:])
            nc.sync.dma_start(gsc[:], GSC[:])

            s_chunks = [sopool.tile([128, CF], f32, tag="so", name=f"s{c}")
                        for c in range(CHUNKS)]
            sq_chunks = [None] * CHUNKS
            dp_chunks = [None] * CHUNKS
            w_chunks = [None] * CHUNKS
            r_chunks = [None] * CHUNKS

            with (
                tc.tile_pool(name="xt", bufs=1) as xtpool,
                tc.tile_pool(name="cnn", bufs=2) as cnnpool,
            ):
                # ---- inputs ----
                xt_sb = xtpool.tile([128, 2, BC], bf16)
                xb_sb = xtpool.tile([128, TILES, 256], bf16)
                XBv = XB[:].rearrange("(t p) n -> p t n", p=128)
                for qq in range(4):
                    qsl = slice(BC // 4 * qq, BC // 4 * (qq + 1))
                    tsl = slice(TILES // 4 * qq, TILES // 4 * (qq + 1))
                    for k in range(2):
                        nc.sync.dma_start(xt_sb[:, k, qsl],
                                          XT[128 * k:128 * (k + 1), qsl])
                    nc.gpsimd.dma_start(xb_sb[:, tsl, :], XBv[:, tsl, :])
                    if qq == 0:
                        for k in range(2):
                            nc.scalar.dma_start(wm[:, k, :],
                                                WM[128 * k:128 * (k + 1), :])

                sp_insts = []
                spEv = spE[:].rearrange("p (t c) -> p c t", c=2)
                spv = sp[:].rearrange("p (t c) -> p c t", c=2)
                p3 = pc3pool.tile([128, 2 * TILES], f32)

                # ======== CNN phase: both supergroups, exp/ln table ========
                for sg in range(NSG):
                    gs_range = range(GPS * sg, GPS * (sg + 1))
                    p2 = pc2pool.tile([128, 512], f32, tag="p2",
                                      name=f"p2sg{sg}")
                    for q, g in enumerate(gs_range):
                        sl = slice(512 * g, 512 * (g + 1))
                        p1 = pc1pool.tile([128, 512], f32, tag="p1",
                                          name=f"p1g{g}")
                        nc.tensor.matmul(p1[:], m1t[:, 0, :],
                                         xt_sb[:, 0, sl],
                                         start=True, stop=False)
                        nc.tensor.matmul(p1[:], m1t[:, 1, :],
                                         xt_sb[:, 1, sl],
                                         start=False, stop=True)
                        eh1 = cnnpool.tile([128, 512], f32, tag="eh1",
                                           name=f"eh1g{g}")
                        nc.scalar.activation(eh1[:], p1[:], AF.Exp,
                                             bias=b2v[:])
                        h1s = cnnpool.tile([128, 512], bf16, tag="h1s",
                                           name=f"h1sg{g}")
                        nc.scalar.activation(h1s[:], eh1[:], AF.Ln, bias=1.0)
                        nc.tensor.matmul(p2[32 * q:32 * (q + 1), :],
                                         m2bd[:], h1s[:],
                                         start=True, stop=True,
                                         tile_position=(0, 32 * q),
                                         skip_group_check=True)
                    eh2 = cnnpool.tile([128, 512], f32, tag="eh2",
                                       name=f"eh2sg{sg}")
                    nc.scalar.activation(eh2[:], p2[:], AF.Exp,
                                         bias=b3v[:])
                    h2s = cnnpool.tile([128, 512], bf16, tag="h2s",
                                       name=f"h2ssg{sg}")
                    nc.scalar.activation(h2s[:], eh2[:], AF.Ln, bias=1.0)
                    for q, g in enumerate(gs_range):
                        for i in range(4):
                            t = 4 * g + i
                            nc.tensor.matmul(
                                p3[:, 2 * t:2 * t + 2],
                                h2s[32 * q:32 * (q + 1),
                                    128 * i:128 * (i + 1)],
                                lwbd[32 * q:32 * (q + 1), :],
                                start=True, stop=True,
                                tile_position=(32 * q, 0),
                                skip_group_check=True)

                    sgt = slice(4 * GPS * sg, 4 * GPS * (sg + 1))
                    sgs = slice(8 * GPS * sg, 8 * GPS * (sg + 1))
                    nc.scalar.activation(spEv[:, 0, sgt],
                                         p3[:, sgs].rearrange(
                                             "p (t c) -> p c t", c=2)[:, 0, :],
                                         AF.Exp, bias=lbm[:])
                    nc.scalar.activation(spEv[:, 1, sgt],
                                         p3[:, sgs].rearrange(
                                             "p (t c) -> p c t", c=2)[:, 1, :],
                                         AF.Exp, bias=lbr[:])
                    sp_i = nc.scalar.activation(sp[:, sgs], spE[:, sgs],
                                                AF.Ln, bias=1.0)
                    sp_insts.append(sp_i)
                    nc.vector.tensor_scalar(gph[:, sgt], spv[:, 0, sgt],
                                            gsc[:], -0.25,
                                            Alu.mult, Alu.add)
                    nc.vector.tensor_scalar(c13p[:, sgt], gph[:, sgt],
                                            2.0 / 3.0, 0.25,
                                            Alu.mult, Alu.add)
                    nc.vector.tensor_scalar(c13x4[:, sgt], gph[:, sgt],
                                            8.0 / 3.0, 1.0,
                                            Alu.mult, Alu.add)

                # ======== main phase: matmuls + per-chunk s/sqe/DMS/sqrt ====
                first_sqrt = None
                last_sqrt_blk = None
                for c in range(CHUNKS):
                    s_c = s_chunks[c]
                    pms = []
                    # identity moves first: eye-stationary, one 256-wide
                    # matmul per tile; start=True resets the whole bank so
                    # the 512-wide W-matmuls below accumulate on a clean
                    # B half.
                    for i in range(CT):
                        t = CT * c + i
                        pm = pmpool.tile([128, 512], f32, tag="pm",
                                         name=f"pm{t}")
                        pms.append(pm)
                        nc.tensor.matmul(pm[:, 0:256], eye[:],
                                         xb_sb[:, t, :],
                                         start=True, stop=False,
                                         skip_group_check=True)
                    for i in range(CT):
                        t = CT * c + i
                        tsl = slice(128 * t, 128 * (t + 1))
                        pm = pms[i]
                        for k in range(2):
                            nc.tensor.matmul(
                                pm[:, 0:512],
                                xt_sb[:, k, tsl], wm[:, k, :],
                                start=False, stop=(k == 1),
                                skip_group_check=True)
                    # elementwise: td, s, per chunk Square, DMS, sqrt
                    for i in range(CT):
                        t = CT * c + i
                        pm = pms[i]
                        osl = slice(256 * i, 256 * (i + 1))
                        td = wvpool.tile([128, 256], f32, tag="td",
                                         name=f"td{t}")
                        nc.vector.tensor_scalar(
                            td[:], pm[:, 256:512],
                            spv[:, 1, t:t + 1], None, Alu.mult)
                        nc.vector.scalar_tensor_tensor(
                            s_c[:, osl], pm[:, 0:256], 1.0 / 3.0,
                            td[:], Alu.add, Alu.add)
                    sqe = sqpool.tile([128, CF], f32, tag="sq",
                                      name=f"sq{c}")
                    sq_chunks[c] = sqe
                    nc.scalar.activation(sqe[:], s_c[:], AF.Square,
                                         bias=cm16[:])
                    dp = dppool.tile([128, CF], f32, tag="dp",
                                     name=f"dp{c}")
                    dp_chunks[c] = dp
                    for i in range(CT):
                        t = CT * c + i
                        osl = slice(256 * i, 256 * (i + 1))
                        nc.vector._custom_dve(
                            COPS['DMS'], out=dp[:, osl],
                            in0=sqe[:, osl],
                            s0=c13p[:, t:t + 1], s1=gph[:, t:t + 1])
                    r = rrpool.tile([128, CF], f32, tag="rr", name=f"r{c}")
                    r_chunks[c] = r
                    for i in range(CT):
                        t = CT * c + i
                        osl = slice(256 * i, 256 * (i + 1))
                        sq_i = nc.scalar.activation(r[:, osl],
                                                    sqe[:, osl],
                                                    AF.Sqrt,
                                                    bias=c13x4[:, t:t + 1],
                                                    scale=4.0)
                        if first_sqrt is None:
                            first_sqrt = sq_i
                            for spi in sp_insts:
                                add_dep_helper(sq_i.ins, spi.ins, sync=True,
                                               reason="sqrt block after NLE")
                        else:
                            add_dep_helper(sq_i.ins, last_sqrt_blk.ins,
                                           sync=False,
                                           reason="chain sqrt block")
                        last_sqrt_blk = sq_i

            # ---- rsqrt block (abs_rsqrt table), WU on DVE ----
            last_rsq = None
            for c in range(CHUNKS):
                dp = dp_chunks[c]
                irs_i = nc.scalar.activation(dp[:], dp[:],
                                             AF.Abs_reciprocal_sqrt)
                if last_rsq is None:
                    add_dep_helper(irs_i.ins, last_sqrt_blk.ins, sync=True,
                                   reason="absrsqrt block after sqrt block")
                else:
                    add_dep_helper(irs_i.ins, last_rsq.ins, sync=False,
                                   reason="chain rsqrt block")
                last_rsq = irs_i
                w = wvpool.tile([128, CF], f32, tag="wv", name=f"w{c}")
                w_chunks[c] = w
                for i in range(CT):
                    t = CT * c + i
                    osl = slice(256 * i, 256 * (i + 1))
                    nc.vector._custom_dve(
                        COPS['WU'], out=w[:, osl], in0=s_chunks[c][:, osl],
                        in1=dp[:, osl], s0=gph[:, t:t + 1], imm2=0.5)

            # ---- trig block: arctan, RGSIN, add (Pool), DMA out ----
            last_at = None
            for c in range(CHUNKS):
                w = w_chunks[c]
                at_i = nc.scalar.activation(w[:], w[:], AF.Arctan)
                if last_at is None:
                    add_dep_helper(at_i.ins, last_rsq.ins, sync=True,
                                   reason="trig block after rsqrt block")
                else:
                    add_dep_helper(at_i.ins, last_at.ins, sync=False,
                                   reason="chain trig block")
                last_at = at_i
                rg = sqpool.tile([128, CF], f32, tag="sq", name=f"rg{c}")
                nc.vector._custom_dve(
                    COPS['RGSIN'], out=rg[:], in0=w[:],
                    in1=r_chunks[c][:],
                    s0=-1.0 / 3.0, s1=1.0 / 162.0, imm2=-1.0 / 29160.0)
                ot = oopool.tile([128, CF], odt, tag="oo", name=f"o{c}")
                if c % 2 == 0:
                    nc.gpsimd.tensor_tensor(ot[:], rg[:], s_chunks[c][:],
                                            Alu.add)
                else:
                    nc.vector.tensor_tensor(ot[:], rg[:], s_chunks[c][:],
                                            Alu.add)
                dview = OUT[512 * c:512 * (c + 1), :].rearrange(
                    "(tt p) n -> p tt n", p=128)
                nc.sync.dma_start(
                    dview, ot[:].rearrange("p (tt n) -> p tt n", n=256))

    nc.compile()
    return nc


def _get_program():
    key = (B_TOTAL, N, N_CORES)
    if key not in _PROG:
        _PROG[key] = _build_program()
    return _PROG[key]


def _host_prep(inputs):
    import ml_dtypes
    bf = ml_dtypes.bfloat16
    x = _np_f32(inputs['x']).reshape(B_TOTAL, N)
    x_b = _np_f32(inputs['x_b']).reshape(B_TOTAL, N)
    m = float(np.asarray(inputs['mass']).reshape(-1)[0])
    gp = float(np.asarray(inputs['gamma_p']).reshape(-1)[0])
    gamma = float(np.log1p(np.exp(gp))) if gp < 30 else gp
    TtT = _np_f32(inputs['TtT'])
    DtD = _np_f32(inputs['DtD'])

    W_A = ((np.eye(N, dtype=np.float32) - np.float32(gamma) * TtT.T)
           / np.float32(3.0 * m)).astype(np.float32)
    W_B = (-np.float32(gamma) * DtD.T / np.float32(3.0 * m)).astype(np.float32)
    WM = np.concatenate([W_A, W_B], axis=1).astype(bf)          # (256,512)

    M1s, M2s, lws = {}, {}, {}
    for tag in ('mu', 'reg'):
        M1s[tag] = _conv_pool_mat(inputs['w2_' + tag], 256)      # (64,256)
        M2s[tag] = _conv_pool_mat(inputs['w3_' + tag], 64)       # (16,64)
        lws[tag] = _np_f32(inputs['lw_' + tag]).reshape(16)
    M1cat = np.concatenate([M1s['mu'], M1s['reg']], axis=0)      # (128,256)
    M1T = np.ascontiguousarray(M1cat.T).astype(bf)               # (256,128)
    M2BD = np.zeros((128, 32), np.float32)
    M2BD[0:64, 0:16] = M2s['mu'].T
    M2BD[64:128, 16:32] = M2s['reg'].T
    M2BD = M2BD.astype(bf)
    LWBD1 = np.zeros((32, 2), np.float32)
    LWBD1[0:16, 0] = lws['mu']
    LWBD1[16:32, 1] = lws['reg']
    LWBD = np.tile(LWBD1, (4, 1)).astype(bf)                     # (128,2)

    def sc(name):
        return float(np.asarray(inputs[name]).reshape(-1)[0])

    B2V = np.full((128, 1), sc('b2_mu'), np.float32)
    B2V[64:] = sc('b2_reg')
    B3V1 = np.full((32, 1), sc('b3_mu'), np.float32)
    B3V1[16:] = sc('b3_reg')
    B3V = np.tile(B3V1, (4, 1))                                  # (128,1)
    LBM = np.full((128, 1), sc('lb_mu'), np.float32)
    LBR = np.full((128, 1), sc('lb_reg'), np.float32)
    GSC = np.full((128, 1), gamma / (m * m), np.float32)

    EYEM = np.eye(128, dtype=np.float32).astype(bf)
    consts = dict(wm=WM, m1t=M1T, m2bd=M2BD, lwbd=LWBD, eye=EYEM,
                  b2v=B2V, b3v=B3V, lbm=LBM, lbr=LBR, gsc=GSC)

    xb3 = (np.float32(gamma / (3.0 * m)) * x_b).astype(bf)
    xbf = x.astype(bf)
    in_maps = []
    for c in range(N_CORES):
        rows = slice(BC * c, BC * (c + 1))
        im = dict(consts)
        im['xt'] = np.ascontiguousarray(xbf[rows].T)
        im['xb'] = np.ascontiguousarray(xb3[rows])
        in_maps.append(im)
    return in_maps, m


def kernel(**inputs) -> np.ndarray:
    from concourse import bass_utils
    nc = _get_program()
    in_maps, m = _host_prep(inputs)
    res = bass_utils.run_bass_kernel_spmd(nc, in_maps,
                                          core_ids=list(range(N_CORES)))
    out = np.concatenate([res.results[c]['out'] for c in range(N_CORES)],
                         axis=0).astype(np.float32)
    if m != 1.0:
        out = (np.float32(m) * out).astype(np.float32)
    return np.ascontiguousarray(out.reshape(B_TOTAL, 1, N))


# revision 12
# speedup vs baseline: 1.0806x; 1.0482x over previous
"""Trainium2 Bass kernel for nn_Block_34067680592489.

Computes, for B=32768 independent signals x[b] (length 256):
  mu,reg = small-CNN(x[b])      (conv5+avgpool4+softplus twice, linear, softplus)
  grad   = TtT x - x_b + reg * DtD x
  x_t    = x - gamma * grad,  gamma = softplus(gamma_p)
  out    = middle root of z^3 -(m+x_t) z^2 + (m x_t - 2 gm) z + gm m,  gm = gamma*mu

Device algorithm (per element, normalized to mass=1; p = s - 1/3 lives in
PSUM directly as the matmul accumulation, s = (1+xt)/3):
  E   = 2p - 1/3  (= 2s - 1)
  E2  = E^2                                  (ACT Square of PSUM, table-free)
  hm4 = E2 + c13x4   (= 4(sqe + c13) = -4p/3... the cubic's -p coeff x4)
  C8  = E*(1.5*hm4 - 0.5*E^2 - 1.5)  (= 8C = -4q)    (fused DVE op)
  D4  = hm4^3 - C8^2 (= 64 D' > 0)                   (fused DVE op)
  z|irs = AbsRsqrt(hm4 | D4)       (ONE table for the whole mid-phase)
  r   = hm4 * z   (= 2 sqrt(hm))
  w   = C8 * irs  (= C/sqrt(D'))
  at  = Arctan(w)                            (trig table, tail block)
  root= s - r*sin(at/3)                      (DVE odd-poly * r, + s16)
The reg * DtD x term is folded into the PE contraction: xr = reg (.) x is
built once on DVE (bf16, 2x mode) from a PE-broadcast of the CNN's reg
output, so no per-tile scaling passes are needed; x_b enters PSUM via one
eye-stationary identity matmul per tile.

Sharding: pure data parallel over batch, 8 cores x 4096 rows.
"""

import numpy as np

B_TOTAL = 32768
N = 256
N_CORES = 8
BC = B_TOTAL // N_CORES      # rows per core
TILES = BC // 128            # 32 batch tiles of 128
CT = 4                       # tiles per elementwise chunk
CHUNKS = TILES // CT         # 8
CF = CT * N                  # chunk free size (1024)

_PROG = {}


def _np_f32(a):
    return np.ascontiguousarray(np.asarray(a, dtype=np.float32))


def _conv_pool_mat(w, L):
    """(L/4, L) matrix implementing conv1d(k=5,pad=2) then avgpool4."""
    taps = np.asarray(w, np.float32).reshape(5)
    C = np.zeros((L, L), np.float32)
    for n in range(L):
        for k in range(5):
            m = n + k - 2
            if 0 <= m < L:
                C[n, m] = taps[k]
    P = np.zeros((L // 4, L), np.float32)
    for i in range(L // 4):
        P[i, 4 * i:4 * i + 4] = 0.25
    return (P @ C).astype(np.float32)


_CUSTOM_OPS = {}


def _get_custom_ops():
    """Register this kernel's fused custom-DVE ops (idempotent).

    CH8:  C8 = (in0*s1 - E^2*imm2 - s1)*E,  E = 2*in1 - s0
          (called with in0=hm4, in1=p(PSUM), s0=1/3, s1=1.5, imm2=0.5)
    DQ:   D4 = in0^3 - in1^2                 (in0=hm4, in1=C8)
    RGSIN: rg = (at*(s0+at^2*(s1+at^2*imm2)))*r   (odd -sin(x/3) poly)
    """
    if _CUSTOM_OPS:
        return _CUSTOM_OPS
    import concourse.dve_ops as dops
    from concourse.dve_spec import (Spec, Src0, Src1, C0, C1, C2, sq,
                                    lower, _has_src1)
    from concourse.dve_uop import DveOpSpec

    def reg(name, spec):
        if name in dops._SUB_OPCODE_FOR_NAME:
            return next(o for o in dops.OPS if o.name == name)
        row = dops._CUSTOM_DVE_ROW_BASE + len(dops.OPS)
        assert row < 0x20
        dops._SUB_OPCODE_FOR_NAME[name] = row
        shas = {}
        for ver in ("v3", "v4"):
            u = lower(spec, ver=ver)
            shas[ver] = DveOpSpec(name=name, opcode=row, uops=u,
                                  rd1_en=_has_src1(spec)).sha(ver)
        op = dops.DveOp(name, spec, subdim=False, uops_sha=shas)
        dops.OPS.append(op)
        dops.CUSTOM_DVE_SPECS[name] = spec
        return op

    import numpy as np_

    _E = (Src1 + Src1) - C0
    _CUSTOM_OPS['CH8'] = reg('ANT_K_CH8', Spec(
        body=((Src0 * C1) - sq(_E) * C2 - C1) * _E,
        reference=lambda in0, in1, s0, s1, imm2:
            (((in0 * s1) - (2.0 * in1 - s0) ** 2 * imm2 - s1)
             * (2.0 * in1 - s0)).astype(np_.float32),
    ))
    _CUSTOM_OPS['DQ'] = reg('ANT_K_DQ', Spec(
        body=(sq(Src0) * Src0) - sq(Src1),
        reference=lambda in0, in1, s0, s1, imm2:
            (in0 ** 2 * in0 - in1 ** 2).astype(np_.float32),
    ))
    _a2 = sq(Src0)
    _CUSTOM_OPS['RGSIN'] = reg('ANT_K_RGSIN', Spec(
        body=(Src0 * (C0 + _a2 * (C1 + _a2 * C2))) * Src1,
        reference=lambda in0, in1, s0, s1, imm2:
            ((in0 * (s0 + in0 * in0 * (s1 + in0 * in0 * imm2))) * in1
             ).astype(np_.float32),
    ))
    return _CUSTOM_OPS


_TABLES_PATCHED = False


def _patch_act_tables():
    """Restrict ACT table-set choice to the sets this kernel uses."""
    global _TABLES_PATCHED
    if _TABLES_PATCHED:
        return
    import concourse.bacc as bacc
    keep = {'natural_log_exp_and_others',
            'abs_reciprocal_sqrt_and_small', 'trig_and_small'}
    orig = bacc.get_activation_tables

    def patched(arch):
        t = orig(arch)
        return {k: (v if k in keep else set()) for k, v in t.items()}

    bacc.get_activation_tables = patched
    _TABLES_PATCHED = True


def _build_program():
    import concourse.bacc as bacc
    import concourse.tile as tile
    import concourse.mybir as mybir
    from concourse.tile import add_dep_helper
    _patch_act_tables()

    dt = mybir.dt
    f32 = dt.float32
    bf16 = dt.bfloat16
    fp16 = dt.float16
    Alu = mybir.AluOpType
    AF = mybir.ActivationFunctionType
    odt = bf16

    COPS = _get_custom_ops()
    nc = bacc.Bacc("TRN2", target_bir_lowering=False, debug=False,
                   num_devices=N_CORES)

    XT = nc.dram_tensor("xt", (256, BC), bf16, kind="ExternalInput")
    XB = nc.dram_tensor("xb", (BC, 256), bf16, kind="ExternalInput")
    EYE = nc.dram_tensor("eye", (128, 128), bf16, kind="ExternalInput")
    WM = nc.dram_tensor("wm", (256, 512), bf16, kind="ExternalInput")
    M1T = nc.dram_tensor("m1t", (256, 128), bf16, kind="ExternalInput")
    M2BD = nc.dram_tensor("m2bd", (128, 32), bf16, kind="ExternalInput")
    LWBD = nc.dram_tensor("lwbd", (128, 2), bf16, kind="ExternalInput")
    B2V = nc.dram_tensor("b2v", (128, 1), f32, kind="ExternalInput")
    B3V = nc.dram_tensor("b3v", (128, 1), f32, kind="ExternalInput")
    LBM = nc.dram_tensor("lbm", (128, 1), f32, kind="ExternalInput")
    LBR = nc.dram_tensor("lbr", (128, 1), f32, kind="ExternalInput")
    GSC = nc.dram_tensor("gsc", (128, 1), f32, kind="ExternalInput")
    ONESR = nc.dram_tensor("onesr", (1, 128), bf16, kind="ExternalInput")
    OUT = nc.dram_tensor("out", (BC, 256), odt, kind="ExternalOutput")

    NSG = 2                          # supergroups
    GPS = CHUNKS // NSG              # groups per supergroup

    with tile.TileContext(nc) as tc:
        with (
            tc.tile_pool(name="const", bufs=1) as cpool,
            tc.tile_pool(name="xin", bufs=1) as xtpool,
            tc.tile_pool(name="hd", bufs=3) as hdpool,
            tc.tile_pool(name="c8", bufs=3) as c8pool,
            tc.tile_pool(name="e2", bufs=2) as e2pool,
            tc.tile_pool(name="s16", bufs=CHUNKS) as s16pool,
            tc.tile_pool(name="rr", bufs=CHUNKS) as rrpool,
            tc.tile_pool(name="wv", bufs=CHUNKS) as wvpool,
            tc.tile_pool(name="rg", bufs=3) as rgpool,
            tc.tile_pool(name="oo", bufs=3) as oopool,
        ):
            # ---- constants into SBUF ----
            wm = cpool.tile([128, 2, 512], bf16)
            m1t = cpool.tile([128, 2, 128], bf16)
            m2bd = cpool.tile([128, 32], bf16)
            lwbd = cpool.tile([128, 2], bf16)
            b2v = cpool.tile([128, 1], f32)
            b3v = cpool.tile([128, 1], f32)
            lbm = cpool.tile([128, 1], f32)
            lbr = cpool.tile([128, 1], f32)
            gsc = cpool.tile([128, 1], f32)
            spE = cpool.tile([128, 2 * TILES], f32)
            sp = cpool.tile([128, 2 * TILES], f32)
            gph = cpool.tile([128, TILES], f32)
            c13x4 = cpool.tile([128, TILES], f32)
            eye = cpool.tile([128, 128], bf16)
            onesr = cpool.tile([1, 128], bf16)
            regc = cpool.tile([128, TILES], bf16)
            regT = cpool.tile([32, 128], bf16)
            regf = cpool.tile([1, BC], bf16)
            regB = cpool.tile([128, BC], bf16)
            epsv = cpool.tile([128, 1], f32)
            nc.vector.memset(epsv[:], 1e-5)
            cm13 = cpool.tile([128, 1], f32)
            nc.vector.memset(cm13[:], -1.0 / 3.0)
            c13v = cpool.tile([128, 1], f32)
            nc.vector.memset(c13v[:], 1.0 / 3.0)
            nc.sync.dma_start(eye[:], EYE[:])
            nc.sync.dma_start(onesr[:], ONESR[:])
            for k in range(2):
                nc.sync.dma_start(m1t[:, k, :], M1T[128 * k:128 * (k + 1), :])
            nc.sync.dma_start(m2bd[:], M2BD[:])
            nc.sync.dma_start(lwbd[:], LWBD[:])
            nc.sync.dma_start(b2v[:], B2V[:])
            nc.sync.dma_start(b3v[:], B3V[:])
            nc.sync.dma_start(lbm[:], LBM[:])
            nc.sync.dma_start(lbr[:], LBR[:])
            nc.sync.dma_start(gsc[:], GSC[:])

            # ---- inputs ----
            xt_sb = xtpool.tile([128, 2, BC], bf16)
            xr_sb = xtpool.tile([128, 2, BC], bf16)
            xb_sb = xtpool.tile([128, TILES, 256], bf16)
            XBv = XB[:].rearrange("(t p) n -> p t n", p=128)
            for qq in range(4):
                qsl = slice(BC // 4 * qq, BC // 4 * (qq + 1))
                tsl = slice(TILES // 4 * qq, TILES // 4 * (qq + 1))
                for k in range(2):
                    nc.sync.dma_start(xt_sb[:, k, qsl],
                                      XT[128 * k:128 * (k + 1), qsl])
                nc.gpsimd.dma_start(xb_sb[:, tsl, :], XBv[:, tsl, :])
                if qq == 0:
                    for k in range(2):
                        nc.scalar.dma_start(wm[:, k, :],
                                            WM[128 * k:128 * (k + 1), :])

            sp_insts = []
            spEv = spE[:].rearrange("p (t c) -> p c t", c=2)
            spv = sp[:].rearrange("p (t c) -> p c t", c=2)

            # ================= CNN + reg-broadcast phase =================
            with (
                tc.tile_pool(name="cnn", bufs=2) as cnnpool,
                tc.tile_pool(name="pc1", bufs=2, space="PSUM") as pc1pool,
                tc.tile_pool(name="pc2", bufs=1, space="PSUM") as pc2pool,
                tc.tile_pool(name="pc3", bufs=1, space="PSUM") as pc3pool,
                tc.tile_pool(name="pcb", bufs=2, space="PSUM") as pcbpool,
            ):
                p3 = pc3pool.tile([128, 2 * TILES], f32)
                for sg in range(NSG):
                    gs_range = range(GPS * sg, GPS * (sg + 1))
                    p2 = pc2pool.tile([128, 512], f32, tag="p2",
                                      name=f"p2sg{sg}")
                    for q, g in enumerate(gs_range):
                        sl = slice(512 * g, 512 * (g + 1))
                        p1 = pc1pool.tile([128, 512], f32, tag="p1",
                                          name=f"p1g{g}")
                        nc.tensor.matmul(p1[:], m1t[:, 0, :],
                                         xt_sb[:, 0, sl],
                                         start=True, stop=False)
                        nc.tensor.matmul(p1[:], m1t[:, 1, :],
                                         xt_sb[:, 1, sl],
                                         start=False, stop=True)
                        eh1 = cnnpool.tile([128, 512], f32, tag="eh1",
                                           name=f"eh1g{g}")
                        nc.scalar.activation(eh1[:], p1[:], AF.Exp,
                                             bias=b2v[:])
                        h1s = cnnpool.tile([128, 512], bf16, tag="h1s",
                                           name=f"h1sg{g}")
                        nc.scalar.activation(h1s[:], eh1[:], AF.Ln, bias=1.0)
                        nc.tensor.matmul(p2[32 * q:32 * (q + 1), :],
                                         m2bd[:], h1s[:],
                                         start=True, stop=True,
                                         tile_position=(0, 32 * q),
                                         skip_group_check=True)
                    eh2 = cnnpool.tile([128, 512], f32, tag="eh2",
                                       name=f"eh2sg{sg}")
                    nc.scalar.activation(eh2[:], p2[:], AF.Exp,
                                         bias=b3v[:])
                    h2s = cnnpool.tile([128, 512], bf16, tag="h2s",
                                       name=f"h2ssg{sg}")
                    nc.scalar.activation(h2s[:], eh2[:], AF.Ln, bias=1.0)
                    for q, g in enumerate(gs_range):
                        for i in range(4):
                            t = 4 * g + i
                            nc.tensor.matmul(
                                p3[:, 2 * t:2 * t + 2],
                                h2s[32 * q:32 * (q + 1),
                                    128 * i:128 * (i + 1)],
                                lwbd[32 * q:32 * (q + 1), :],
                                start=True, stop=True,
                                tile_position=(32 * q, 0),
                                skip_group_check=True)

                    sgt = slice(4 * GPS * sg, 4 * GPS * (sg + 1))
                    sgs = slice(8 * GPS * sg, 8 * GPS * (sg + 1))
                    nc.scalar.activation(spEv[:, 0, sgt],
                                         p3[:, sgs].rearrange(
                                             "p (t c) -> p c t", c=2)[:, 0, :],
                                         AF.Exp, bias=lbm[:])
                    nc.scalar.activation(spEv[:, 1, sgt],
                                         p3[:, sgs].rearrange(
                                             "p (t c) -> p c t", c=2)[:, 1, :],
                                         AF.Exp, bias=lbr[:])
                    sp_i = nc.scalar.activation(sp[:, sgs], spE[:, sgs],
                                                AF.Ln, bias=1.0)
                    sp_insts.append(sp_i)
                    nc.vector.tensor_scalar(gph[:, sgt], spv[:, 0, sgt],
                                            gsc[:], -0.25,
                                            Alu.mult, Alu.add)
                    nc.vector.tensor_scalar(c13x4[:, sgt], gph[:, sgt],
                                            8.0 / 3.0, 1.0,
                                            Alu.mult, Alu.add)
                    # bf16 copy of reg for the PE transpose
                    nc.vector.tensor_scalar(regc[:, sgt], spv[:, 1, sgt],
                                            1.0, None, Alu.mult)

                # reg (per batch row, [128, T] layout) -> regB [128, BC]
                # broadcast along partitions, via PE transpose + 1-contract
                # broadcast matmuls.
                pT = pcbpool.tile([32, 128], f32, tag="pt", name="pT")
                nc.tensor.matmul(pT[:], regc[:], eye[:],
                                 start=True, stop=True)
                nc.vector.tensor_scalar(regT[:], pT[:], 1.0, None, Alu.mult)
                nc.sync.dma_start(
                    regf[:].rearrange("o (t n) -> o t n", n=128),
                    regT[:].rearrange("t n -> t () n"))
                for t in range(TILES):
                    bb = t // 4
                    pB = (pcbpool.tile([128, 512], f32, tag="pb",
                                       name=f"pB{bb}")
                          if t % 4 == 0 else pB)
                    nc.tensor.matmul(pB[:, 128 * (t % 4):128 * (t % 4 + 1)],
                                     onesr[:], regf[0:1, 128 * t:128 * (t + 1)],
                                     start=(t % 4 == 0), stop=(t % 4 == 3),
                                     skip_group_check=True)
                    if t % 4 == 3:
                        nc.vector.tensor_scalar(
                            regB[:, 512 * bb:512 * (bb + 1)], pB[:],
                            1.0, None, Alu.mult)

                # xr = reg (.) x  (bf16, DVE 2x mode)
                for k in range(2):
                    nc.vector.tensor_tensor(xr_sb[:, k, :], xt_sb[:, k, :],
                                            regB[:], Alu.mult)

            # ================= main phase =================
            s16_chunks = [None] * CHUNKS
            r_chunks = [None] * CHUNKS
            w_chunks = [None] * CHUNKS
            last_zd = None
            with tc.tile_pool(name="pm", bufs=3, space="PSUM") as pmpool:
                for c in range(CHUNKS):
                    pm = pmpool.tile([128, CF], f32, tag="pm", name=f"pm{c}")
                    # identity moves first (eye stationary; start=True resets
                    # each 512-col bank, so issue per-bank leader first)
                    for i in range(CT):
                        t = CT * c + i
                        nc.tensor.matmul(pm[:, 256 * i:256 * (i + 1)],
                                         eye[:], xb_sb[:, t, :],
                                         start=(i % 2 == 0), stop=False,
                                         skip_group_check=True)
                    for i in range(CT):
                        t = CT * c + i
                        tsl = slice(128 * t, 128 * (t + 1))
                        osl = slice(256 * i, 256 * (i + 1))
                        for k in range(2):
                            nc.tensor.matmul(
                                pm[:, osl], xt_sb[:, k, tsl],
                                wm[:, k, 0:256],
                                start=False, stop=False,
                                skip_group_check=True)
                            nc.tensor.matmul(
                                pm[:, osl], xr_sb[:, k, tsl],
                                wm[:, k, 256:512],
                                start=False, stop=(k == 1),
                                skip_group_check=True)
                    # E2 = (2p - 1/3)^2  [ACT, table-free]
                    e2 = e2pool.tile([128, CF], fp16, tag="e2",
                                     name=f"e2c{c}")
                    nc.scalar.activation(e2[:], pm[:], AF.Square,
                                         scale=2.0, bias=cm13[:])
                    # hm4 = E2 + c13x4 (per-tile scalar; fp16 4x mode)
                    hd = hdpool.tile([128, 2, CF], fp16, tag="hd",
                                     name=f"hd{c}")
                    for i in range(CT):
                        t = CT * c + i
                        osl = slice(256 * i, 256 * (i + 1))
                        nc.vector.tensor_scalar(hd[:, 0, osl], e2[:, osl],
                                                c13x4[:, t:t + 1], None,
                                                Alu.add)
                    # C8, D4
                    c8 = c8pool.tile([128, CF], fp16, tag="c8",
                                     name=f"c8c{c}")
                    nc.vector._custom_dve(
                        COPS['CH8'], out=c8[:], in0=hd[:, 0, :], in1=pm[:],
                        s0=1.0 / 3.0, s1=1.5, imm2=0.5)
                    nc.vector._custom_dve(
                        COPS['DQ'], out=hd[:, 1, :], in0=hd[:, 0, :],
                        in1=c8[:])
                    # z | irs = AbsRsqrt(hm4 | D4)  [one 2048-wide ACT op]
                    zd = rgpool.tile([128, 2, CF], fp16, tag="zd",
                                     name=f"zd{c}")
                    zd_i = nc.scalar.activation(
                        zd[:].rearrange("p a b -> p (a b)"),
                        hd[:].rearrange("p a b -> p (a b)"),
                        AF.Abs_reciprocal_sqrt, bias=epsv[:])
                    if last_zd is None:
                        for spi in sp_insts:
                            add_dep_helper(zd_i.ins, spi.ins, sync=True,
                                           reason="absrsqrt after NLE table")
                    else:
                        add_dep_helper(zd_i.ins, last_zd.ins, sync=False,
                                       reason="chain zd block")
                    last_zd = zd_i
                    # r = hm4*z,  w = C8*irs  (fp16 2x tt)
                    r = rrpool.tile([128, CF], fp16, tag="rr", name=f"r{c}")
                    r_chunks[c] = r
                    nc.vector.tensor_tensor(r[:], hd[:, 0, :], zd[:, 0, :],
                                            Alu.mult)
                    w = wvpool.tile([128, CF], fp16, tag="wv", name=f"w{c}")
                    w_chunks[c] = w
                    nc.vector.tensor_tensor(w[:], c8[:], zd[:, 1, :],
                                            Alu.mult)
                    # s16 = p + 1/3 (frees the PSUM bank pair; GPSIMD can't
                    # read PSUM, so split between DVE and ACT-Identity)
                    s16 = s16pool.tile([128, CF], fp16, tag="s16",
                                       name=f"s16c{c}")
                    s16_chunks[c] = s16
                    if c % 2 == 0:
                        nc.vector.tensor_scalar(s16[:], pm[:], 1.0 / 3.0,
                                                None, Alu.add)
                    else:
                        nc.scalar.activation(s16[:], pm[:], AF.Identity,
                                             bias=c13v[:])

            # ---- trig tail: arctan block, RGSIN, add, DMA out ----
            last_at = None
            for c in range(CHUNKS):
                w = w_chunks[c]
                at_i = nc.scalar.activation(w[:], w[:], AF.Arctan)
                if last_at is None:
                    add_dep_helper(at_i.ins, last_zd.ins, sync=True,
                                   reason="trig block after absrsqrt block")
                else:
                    add_dep_helper(at_i.ins, last_at.ins, sync=False,
                                   reason="chain trig block")
                last_at = at_i
                rg = rgpool.tile([128, CF], fp16, tag="rg", name=f"rgc{c}")
                nc.vector._custom_dve(
                    COPS['RGSIN'], out=rg[:], in0=w[:],
                    in1=r_chunks[c][:],
                    s0=-1.0 / 3.0, s1=1.0 / 162.0, imm2=-1.0 / 29160.0)
                ot = oopool.tile([128, CF], odt, tag="oo", name=f"o{c}")
                nc.vector.tensor_tensor(ot[:], rg[:], s16_chunks[c][:],
                                        Alu.add)
                dview = OUT[512 * c:512 * (c + 1), :].rearrange(
                    "(tt p) n -> p tt n", p=128)
                nc.sync.dma_start(
                    dview, ot[:].rearrange("p (tt n) -> p tt n", n=256))

    nc.compile()
    return nc


def _get_program():
    key = (B_TOTAL, N, N_CORES)
    if key not in _PROG:
        _PROG[key] = _build_program()
    return _PROG[key]


def _host_prep(inputs):
    import ml_dtypes
    bf = ml_dtypes.bfloat16
    x = _np_f32(inputs['x']).reshape(B_TOTAL, N)
    x_b = _np_f32(inputs['x_b']).reshape(B_TOTAL, N)
    m = float(np.asarray(inputs['mass']).reshape(-1)[0])
    gp = float(np.asarray(inputs['gamma_p']).reshape(-1)[0])
    gamma = float(np.log1p(np.exp(gp))) if gp < 30 else gp
    TtT = _np_f32(inputs['TtT'])
    DtD = _np_f32(inputs['DtD'])

    W_A = ((np.eye(N, dtype=np.float32) - np.float32(gamma) * TtT.T)
           / np.float32(3.0 * m)).astype(np.float32)
    W_B = (-np.float32(gamma) * DtD.T / np.float32(3.0 * m)).astype(np.float32)
    WM = np.concatenate([W_A, W_B], axis=1).astype(bf)          # (256,512)

    M1s, M2s, lws = {}, {}, {}
    for tag in ('mu', 'reg'):
        M1s[tag] = _conv_pool_mat(inputs['w2_' + tag], 256)      # (64,256)
        M2s[tag] = _conv_pool_mat(inputs['w3_' + tag], 64)       # (16,64)
        lws[tag] = _np_f32(inputs['lw_' + tag]).reshape(16)
    M1cat = np.concatenate([M1s['mu'], M1s['reg']], axis=0)      # (128,256)
    M1T = np.ascontiguousarray(M1cat.T).astype(bf)               # (256,128)
    M2BD = np.zeros((128, 32), np.float32)
    M2BD[0:64, 0:16] = M2s['mu'].T
    M2BD[64:128, 16:32] = M2s['reg'].T
    M2BD = M2BD.astype(bf)
    LWBD1 = np.zeros((32, 2), np.float32)
    LWBD1[0:16, 0] = lws['mu']
    LWBD1[16:32, 1] = lws['reg']
    LWBD = np.tile(LWBD1, (4, 1)).astype(bf)                     # (128,2)

    def sc(name):
        return float(np.asarray(inputs[name]).reshape(-1)[0])

    B2V = np.full((128, 1), sc('b2_mu'), np.float32)
    B2V[64:] = sc('b2_reg')
    B3V1 = np.full((32, 1), sc('b3_mu'), np.float32)
    B3V1[16:] = sc('b3_reg')
    B3V = np.tile(B3V1, (4, 1))                                  # (128,1)
    LBM = np.full((128, 1), sc('lb_mu'), np.float32)
    LBR = np.full((128, 1), sc('lb_reg'), np.float32)
    GSC = np.full((128, 1), gamma / (m * m), np.float32)

    EYEM = np.eye(128, dtype=np.float32).astype(bf)
    ONES = np.ones((1, 128), np.float32).astype(bf)
    consts = dict(wm=WM, m1t=M1T, m2bd=M2BD, lwbd=LWBD, eye=EYEM,
                  onesr=ONES, b2v=B2V, b3v=B3V, lbm=LBM, lbr=LBR, gsc=GSC)

    xb3 = (np.float32(gamma / (3.0 * m)) * x_b).astype(bf)
    xbf = x.astype(bf)
    in_maps = []
    for c in range(N_CORES):
        rows = slice(BC * c, BC * (c + 1))
        im = dict(consts)
        im['xt'] = np.ascontiguousarray(xbf[rows].T)
        im['xb'] = np.ascontiguousarray(xb3[rows])
        in_maps.append(im)
    return in_maps, m


def kernel(**inputs) -> np.ndarray:
    from concourse import bass_utils
    nc = _get_program()
    in_maps, m = _host_prep(inputs)
    res = bass_utils.run_bass_kernel_spmd(nc, in_maps,
                                          core_ids=list(range(N_CORES)))
    out = np.concatenate([res.results[c]['out'] for c in range(N_CORES)],
                         axis=0).astype(np.float32)
    if m != 1.0:
        out = (np.float32(m) * out).astype(np.float32)
    return np.ascontiguousarray(out.reshape(B_TOTAL, 1, N))
